# revision 6
# baseline (speedup 1.0000x reference)
"""Continuous Game-of-Life Trainium2 kernel.

Reference computation (per batch image, cyclic 3x3 stencil):
    around = 8-neighbor sum of x (torus wrap)
    survive = sigmoid(10(around-1.5)) * sigmoid(10(3.5-around))
    birth   = sigmoid(10(around-2.5)) * sigmoid(10(3.5-around))
    out     = x*survive + (1-x)*birth

Algebra: with BETA=10 the transitions are >= 1.0 apart, so
    out ~= x*(s15-s25) + (s25-s35),  s_c = sigmoid(10*around-10c)
Both sigmoid differences are copies of the same unit bump
    g(u) = sigmoid(10u+5) - sigmoid(10u-5) = sigmoid(5-10|u|) + O(6.7e-3):
        s15-s25 = g(around-2),   s25-s35 = g(around-3)
so ONE double-width sigmoid pass over [|t-2| , |t-3|] replaces three
full-width sigmoid passes (ScalarE was the bottleneck: 3 passes
~204us/core -> extraction pass + double-width pass ~190us... split so
the extraction doubles as the PSUM->SBUF fp16 conversion).  Measured
rel err of the whole fp16 pipeline: ~2.4e-3 (harness gate 2e-2).

Memory: x is cast to fp16 on the HOST, so the device reads 16MB/core
instead of 32MB (the stencil and final multiply use fp16 x anyway).
In+out HBM traffic: 32MB/core.  Input DMA moves off the gpsimd SWDGE
cast path onto hardware-DGE (sync) queues, freeing the gpsimd engine
to run the final add.

Engine mapping per 126-row strip of a 2048x2048 image:
  - DMA in (sync HWDGE, fp16): 128 rows, prefetched one strip ahead.
  - TensorE: 8-neighbor sum via banded matmuls in PSUM (3 passes over W:
    no-center band on center columns + full band on column-shifted
    views; W-wrap via two 1-col edge matmuls).
  - ScalarE: zw0 = Copy(ps - 2) -> fp16 SBUF (= t-2; the PSUM read).
  - DVE: zw1 = zw0 - 1 (4x tensor_scalar)              (= t-3)
  - DVE: nzw = -zw (4x tensor_scalar, double-width)
  - DVE: zq = max(zw, nzw) (2x tensor_tensor, double-width) (= |t-c|)
  - ScalarE: ONE wide sigmoid sw = sigmoid(5 - 10*zq) -> [d1 | d2].
  - DVE: m = x*d1 (2x); GpSimd: out = m + d2.
  - DMA out (sync HWDGE): fp16, host upcasts to fp32.
The scalar engine's two ops per strip are software-pipelined (the wide
sigmoid of strip t-1 is emitted after the extraction of strip t) so
ScalarE never stalls on the intervening DVE ops of the same strip.

Sharding: data-parallel over batch: 16 images -> 8 cores x 2 images.
Torus wrap is per-image so there is no cross-core halo.
"""

import numpy as np

B, H, W = 16, 2048, 2048
N_CORES = 8
B_PER = B // N_CORES  # 2 images per core
STRIDE = 126  # output rows per strip (128 input rows incl. 1-row halos)
N_STRIPS = (H + STRIDE - 1) // STRIDE  # 17
NBANKS = W // 512  # PSUM banks per strip

_cached = {}


def _band_matrices(m, dtype=np.float16):
    """[m+2, m] stationary operands for the vertical taps.

    Tile layout: partitions 0..m-1 hold image rows r0..r0+m-1 (the cells),
    partition m holds the bottom halo row r0+m, partition m+1 holds the top
    halo row r0-1.  For output row p the vertical neighbors are partitions
    p-1 (or m+1 when p==0) and p+1.

    m0[k, p] = 1 for the two vertical neighbors (no center),
    m1[k, p] = 1 for the full 3-tap (used on the column-shifted views).
    """
    m0 = np.zeros((m + 2, m), dtype)
    m1 = np.zeros((m + 2, m), dtype)
    for p in range(m):
        up = m + 1 if p == 0 else p - 1
        m0[up, p] = 1.0
        m0[p + 1, p] = 1.0
        m1[up, p] = 1.0
        m1[p, p] = 1.0
        m1[p + 1, p] = 1.0
    return m0, m1


def _build():
    key = (B_PER, H, W, STRIDE)
    if key in _cached:
        return _cached[key]

    import concourse.mybir as mybir
    from concourse.bacc import Bacc
    from concourse.tile import TileContext

    KROWS = STRIDE + 2
    f32 = mybir.dt.float32
    f16 = mybir.dt.float16
    Sig = mybir.ActivationFunctionType.Sigmoid
    Cpy = mybir.ActivationFunctionType.Copy
    Alu = mybir.AluOpType

    nc = Bacc(trn_type="TRN2")
    x_d = nc.dram_tensor("x", [B_PER, H, W], f16, kind="ExternalInput")
    y_d = nc.dram_tensor("y", [B_PER, H, W], f16, kind="ExternalOutput")

    consts = {}
    for m in sorted({STRIDE, H - STRIDE * (N_STRIPS - 1)}):
        m0_np, m1_np = _band_matrices(m)
        consts[m] = (
            nc.inline_tensor(m0_np, f"m0_const_{m}"),
            nc.inline_tensor(m1_np, f"m1_const_{m}"),
        )

    strips = []
    for b in range(B_PER):
        for t in range(N_STRIPS):
            r0 = t * STRIDE
            strips.append((b, r0, min(STRIDE, H - r0)))

    with TileContext(nc) as tc:
        with (
            tc.tile_pool(name="wpool", bufs=1) as wpool,
            tc.tile_pool(name="xpool", bufs=6) as xpool,
            tc.tile_pool(name="zpool", bufs=3) as zpool,
            tc.tile_pool(name="npool", bufs=3) as npool,
            tc.tile_pool(name="qpool", bufs=3) as qpool,
            tc.tile_pool(name="spool", bufs=3) as spool,
            tc.tile_pool(name="mpool", bufs=3) as mpool,
            tc.tile_pool(name="opool", bufs=4) as opool,
            tc.tile_pool(name="ppool", bufs=2, space="PSUM") as ppool,
        ):
            bands = {}
            for m, (m0_d, m1_d) in consts.items():
                m0 = wpool.tile([m + 2, m], f16, name=f"m0_{m}")
                m1 = wpool.tile([m + 2, m], f16, name=f"m1_{m}")
                nc.sync.dma_start(out=m0[:], in_=m0_d[:])
                nc.sync.dma_start(out=m1[:], in_=m1_d[:])
                bands[m] = (m0, m1)

            # sigmoid bias must be an AP, not an immediate
            b5 = wpool.tile([128, 1], f32)
            nc.vector.memset(b5[:], 5.0)

            def issue_dma(b, r0, M, xt):
                # cells + bottom halo + top halo, all fp16 on HWDGE (sync)
                if r0 + M < H:
                    nc.sync.dma_start(
                        out=xt[0 : M + 1, :], in_=x_d[b, r0 : r0 + M + 1, :]
                    )
                else:
                    nc.sync.dma_start(out=xt[0:M, :], in_=x_d[b, r0:H, :])
                    nc.sync.dma_start(out=xt[M : M + 1, :], in_=x_d[b, 0:1, :])
                rtop = (r0 - 1) % H
                nc.sync.dma_start(
                    out=xt[M + 1 : M + 2, :], in_=x_d[b, rtop : rtop + 1, :]
                )

            xts = {}

            def prefetch(si):
                if si < len(strips):
                    pb, pr0, pM = strips[si]
                    t = xpool.tile([KROWS, W], f16, tag="xt", name=f"xt_{si}")
                    issue_dma(pb, pr0, pM, t)
                    xts[si] = t

            def emit_back(b, r0, M, xt, zq):
                # ONE wide sigmoid: [d1 | d2] = sigmoid(5 - 10*|t-c|)
                sw = spool.tile([STRIDE, 2 * W], f16, tag="sw", name="sw")
                nc.scalar.activation(sw[:M], zq[:M], Sig, bias=b5[:M], scale=-10.0)
                # out = x*d1 + d2  (mul on DVE, add on gpsimd)
                mt = mpool.tile([STRIDE, W], f16, tag="m", name="mt")
                ot = opool.tile([STRIDE, W], f16, tag="o", name="ot")
                nc.vector.tensor_mul(out=mt[:M], in0=xt[:M, :], in1=sw[:M, 0:W])
                nc.gpsimd.tensor_add(out=ot[:M], in0=mt[:M], in1=sw[:M, W : 2 * W])
                nc.sync.dma_start(out=y_d[b, r0 : r0 + M, :], in_=ot[:M])

            # Two-stage software pipeline: FRONT(si) = input prefetch,
            # matmuls, PSUM extraction, |t-c| args; BACK(si) = wide
            # sigmoid, combine, output DMA, emitted one iteration later.
            prefetch(0)
            back = [None]

            for si, (b, r0, M) in enumerate(strips):
                k = M + 2
                m0, m1 = bands[M]
                xt = xts.pop(si)
                prefetch(si + 1)

                ps = ppool.tile([STRIDE, W], f32, tag="ps")
                m0s = m0[:k, :M]
                m1s = m1[:k, :M]

                # Pre-touch: a 1x1 matmul absorbs the PSUM-release wait
                # (Matmult carries at most ONE sync wait; without this,
                # wait-merging couples strip t to strip t-1's consumers
                # and serializes PE behind them).
                nc.tensor.matmul(
                    ps[:1, 0:1], b5[:1, :1], b5[:1, :1], start=True, stop=True
                )

                # around = sum of 8 neighbors, accumulated in PSUM.
                for nb in range(NBANKS):
                    c0 = nb * 512
                    c1 = c0 + 512
                    nc.tensor.matmul(
                        ps[:M, c0:c1], m0s, xt[:k, c0:c1], start=True, stop=False
                    )
                    if nb == 0:
                        nc.tensor.matmul(
                            ps[:M, 1:512], m1s, xt[:k, 0:511],
                            start=False, stop=False,
                        )
                        nc.tensor.matmul(
                            ps[:M, 0:1], m1s, xt[:k, W - 1 : W],
                            start=False, stop=False,
                        )
                    else:
                        nc.tensor.matmul(
                            ps[:M, c0:c1], m1s, xt[:k, c0 - 1 : c1 - 1],
                            start=False, stop=False,
                        )
                    if nb == NBANKS - 1:
                        nc.tensor.matmul(
                            ps[:M, c0 : W - 1], m1s, xt[:k, c0 + 1 : W],
                            start=False, stop=False,
                        )
                        nc.tensor.matmul(
                            ps[:M, W - 1 : W], m1s, xt[:k, 0:1],
                            start=False, stop=True,
                        )
                    else:
                        nc.tensor.matmul(
                            ps[:M, c0:c1], m1s, xt[:k, c0 + 1 : c1 + 1],
                            start=False, stop=True,
                        )

                # zw = [t-2 | t-3] fp16.  Half 0 straight off PSUM on the
                # scalar engine (doubles as the PSUM->SBUF conversion),
                # half 1 derived on DVE at 4x rate.
                zw = zpool.tile([STRIDE, 2 * W], f16, tag="zw")
                nc.scalar.activation(zw[:M, 0:W], ps[:M], Cpy, bias=-2.0)
                nc.vector.tensor_scalar_sub(
                    out=zw[:M, W : 2 * W], in0=zw[:M, 0:W], scalar1=1.0
                )

                # zq = |zw| = max(zw, -zw): 4x negate + 2x max (the fused
                # scalar_tensor_tensor form only runs at 1x, so two ops
                # are faster)
                nzw = npool.tile([STRIDE, 2 * W], f16, tag="nzw")
                nc.vector.tensor_scalar_mul(out=nzw[:M], in0=zw[:M], scalar1=-1.0)
                zq = qpool.tile([STRIDE, 2 * W], f16, tag="zq")
                nc.vector.tensor_tensor(
                    out=zq[:M], in0=zw[:M], in1=nzw[:M], op=Alu.max
                )

                back.append((b, r0, M, xt, zq))
                prev = back.pop(0)
                if prev is not None:
                    emit_back(*prev)

            last = back.pop(0)
            if last is not None:
                emit_back(*last)

    nc.compile()
    _cached[key] = nc
    return nc


def run(x, trace=False):
    """Run the SPMD kernel on 8 cores. Returns (out_fp32, BassKernelResults)."""
    from concourse.bass_utils import run_bass_kernel_spmd

    nc = _build()
    x = np.asarray(x, dtype=np.float32)
    assert x.shape == (B, H, W), x.shape
    x16 = x.astype(np.float16)  # host-side cast: halves input HBM traffic
    in_maps = [{"x": x16[B_PER * c : B_PER * (c + 1)]} for c in range(N_CORES)]
    res = run_bass_kernel_spmd(nc, in_maps, core_ids=list(range(N_CORES)), trace=trace)
    out = np.concatenate(
        [res.results[c]["y"].astype(np.float32) for c in range(N_CORES)], axis=0
    )
    return out, res


def kernel(x):
    out, _ = run(x, trace=False)
    return out


# revision 10
# speedup vs baseline: 2.0085x; 2.0085x over previous
"""Continuous Game-of-Life Trainium2 kernel.

Reference computation (per batch image, cyclic 3x3 stencil):
    around = 8-neighbor sum of x (torus wrap)
    survive = sigmoid(10(around-1.5)) * sigmoid(10(3.5-around))
    birth   = sigmoid(10(around-2.5)) * sigmoid(10(3.5-around))
    out     = x*survive + (1-x)*birth

Algebra: with BETA=10 the transitions are >= 1.0 apart, so
    out ~= x*(s15-s25) + (s25-s35),  s_c = sigmoid(10*around-10c)
Both sigmoid differences are copies of the same unit bump
    g(u) = sigmoid(10u+5) - sigmoid(10u-5) = sigmoid(5-10|u|) + O(6.7e-3):
        s15-s25 = g(around-2),   s25-s35 = g(around-3)
so ONE double-width sigmoid pass over [|t-2| , |t-3|] replaces three
full-width sigmoid passes (ScalarE was the bottleneck: 3 passes
~204us/core -> extraction pass + double-width pass ~190us... split so
the extraction doubles as the PSUM->SBUF fp16 conversion).  Measured
rel err of the whole fp16 pipeline: ~2.4e-3 (harness gate 2e-2).

Memory: x is cast to fp16 on the HOST, so the device reads 16MB/core
instead of 32MB (the stencil and final multiply use fp16 x anyway).
In+out HBM traffic: 32MB/core.  Input stays on gpsimd SWDGE (its
descriptors fan out over all 16 DMA engines; a HWDGE DRAM->SBUF dma is
pinned to one engine at 22.5 GB/s) but no longer casts.

Engine mapping per 126-row strip of a 2048x2048 image:
  - DMA in (gpsimd SWDGE, fp16): 128 rows, prefetched one strip ahead.
  - TensorE: 8-neighbor sum via banded matmuls in PSUM (3 passes over W:
    no-center band on center columns + full band on column-shifted
    views; W-wrap via two 1-col edge matmuls).
  - ScalarE: zw0 = Copy(ps - 2) -> fp16 SBUF (= t-2; the PSUM read).
  - DVE: zw1 = zw0 - 1 (4x tensor_scalar)              (= t-3)
  - DVE: nzw = -zw (4x tensor_scalar, double-width)
  - DVE: zq = max(zw, nzw) (2x tensor_tensor, double-width) (= |t-c|)
  - ScalarE: ONE wide sigmoid sw = sigmoid(5 - 10*zq) -> [d1 | d2].
  - DVE: m = x*d1 (2x); out = m + d2 split DVE / GpSimd by columns.
  - DMA out (sync HWDGE): fp16, host upcasts to fp32.
The scalar engine's two ops per strip are software-pipelined (the wide
sigmoid of strip t-1 is emitted after the extraction of strip t) so
ScalarE never stalls on the intervening DVE ops of the same strip.

Sharding: data-parallel over batch: 16 images -> 8 cores x 2 images.
Torus wrap is per-image so there is no cross-core halo.
"""

import numpy as np

B, H, W = 16, 2048, 2048
N_CORES = 8
B_PER = B // N_CORES  # 2 images per core
STRIDE = 126  # output rows per strip (128 input rows incl. 1-row halos)
N_STRIPS = (H + STRIDE - 1) // STRIDE  # 17
NBANKS = W // 512  # PSUM banks per strip

_cached = {}


def _band_matrices(m, dtype=np.float16):
    """[m+2, m] stationary operands for the vertical taps.

    Tile layout: partitions 0..m-1 hold image rows r0..r0+m-1 (the cells),
    partition m holds the bottom halo row r0+m, partition m+1 holds the top
    halo row r0-1.  For output row p the vertical neighbors are partitions
    p-1 (or m+1 when p==0) and p+1.

    m0[k, p] = 1 for the two vertical neighbors (no center),
    m1[k, p] = 1 for the full 3-tap (used on the column-shifted views).
    """
    m0 = np.zeros((m + 2, m), dtype)
    m1 = np.zeros((m + 2, m), dtype)
    for p in range(m):
        up = m + 1 if p == 0 else p - 1
        m0[up, p] = 1.0
        m0[p + 1, p] = 1.0
        m1[up, p] = 1.0
        m1[p, p] = 1.0
        m1[p + 1, p] = 1.0
    return m0, m1


def _build():
    key = (B_PER, H, W, STRIDE)
    if key in _cached:
        return _cached[key]

    import concourse.mybir as mybir
    from concourse.bacc import Bacc
    from concourse.tile import TileContext

    KROWS = STRIDE + 2
    f32 = mybir.dt.float32
    f16 = mybir.dt.float16
    Sig = mybir.ActivationFunctionType.Sigmoid
    Cpy = mybir.ActivationFunctionType.Copy
    Alu = mybir.AluOpType

    nc = Bacc(trn_type="TRN2")
    x_d = nc.dram_tensor("x", [B_PER, H, W], f16, kind="ExternalInput")
    y_d = nc.dram_tensor("y", [B_PER, H, W], f16, kind="ExternalOutput")

    consts = {}
    for m in sorted({STRIDE, H - STRIDE * (N_STRIPS - 1)}):
        m0_np, m1_np = _band_matrices(m)
        consts[m] = (
            nc.inline_tensor(m0_np, f"m0_const_{m}"),
            nc.inline_tensor(m1_np, f"m1_const_{m}"),
        )

    strips = []
    for b in range(B_PER):
        for t in range(N_STRIPS):
            r0 = t * STRIDE
            strips.append((b, r0, min(STRIDE, H - r0)))

    with TileContext(nc) as tc:
        with (
            tc.tile_pool(name="wpool", bufs=1) as wpool,
            tc.tile_pool(name="xpool", bufs=6) as xpool,
            tc.tile_pool(name="zpool", bufs=3) as zpool,
            tc.tile_pool(name="npool", bufs=3) as npool,
            tc.tile_pool(name="qpool", bufs=3) as qpool,
            tc.tile_pool(name="spool", bufs=3) as spool,
            tc.tile_pool(name="mpool", bufs=3) as mpool,
            tc.tile_pool(name="opool", bufs=4) as opool,
            tc.tile_pool(name="ppool", bufs=2, space="PSUM") as ppool,
        ):
            bands = {}
            for m, (m0_d, m1_d) in consts.items():
                m0 = wpool.tile([m + 2, m], f16, name=f"m0_{m}")
                m1 = wpool.tile([m + 2, m], f16, name=f"m1_{m}")
                nc.sync.dma_start(out=m0[:], in_=m0_d[:])
                nc.sync.dma_start(out=m1[:], in_=m1_d[:])
                bands[m] = (m0, m1)

            # sigmoid bias must be an AP, not an immediate
            b5 = wpool.tile([128, 1], f32)
            nc.vector.memset(b5[:], 5.0)

            def issue_dma(b, r0, M, xt):
                # cells + bottom halo + top halo, fp16.  gpsimd SWDGE: its
                # descriptors fan out over all 16 DMA engines (a HWDGE
                # DRAM->SBUF dma is pinned to ONE engine, 22.5 GB/s).
                if r0 + M < H:
                    nc.gpsimd.dma_start(
                        out=xt[0 : M + 1, :], in_=x_d[b, r0 : r0 + M + 1, :]
                    )
                else:
                    nc.gpsimd.dma_start(out=xt[0:M, :], in_=x_d[b, r0:H, :])
                    nc.gpsimd.dma_start(out=xt[M : M + 1, :], in_=x_d[b, 0:1, :])
                rtop = (r0 - 1) % H
                nc.gpsimd.dma_start(
                    out=xt[M + 1 : M + 2, :], in_=x_d[b, rtop : rtop + 1, :]
                )

            xts = {}

            def prefetch(si):
                if si < len(strips):
                    pb, pr0, pM = strips[si]
                    t = xpool.tile([KROWS, W], f16, tag="xt", name=f"xt_{si}")
                    issue_dma(pb, pr0, pM, t)
                    xts[si] = t

            CSPLIT = 640  # add-columns done by DVE; the rest go to gpsimd

            def emit_back(b, r0, M, xt, zq):
                # ONE wide sigmoid: [d1 | d2] = sigmoid(5 - 10*|t-c|)
                sw = spool.tile([STRIDE, 2 * W], f16, tag="sw", name="sw")
                nc.scalar.activation(sw[:M], zq[:M], Sig, bias=b5[:M], scale=-10.0)
                # out = x*d1 + d2  (mul on DVE; add split DVE/gpsimd)
                mt = mpool.tile([STRIDE, W], f16, tag="m", name="mt")
                ot = opool.tile([STRIDE, W], f16, tag="o", name="ot")
                nc.vector.tensor_mul(out=mt[:M], in0=xt[:M, :], in1=sw[:M, 0:W])
                nc.vector.tensor_add(
                    out=ot[:M, 0:CSPLIT],
                    in0=mt[:M, 0:CSPLIT],
                    in1=sw[:M, W : W + CSPLIT],
                )
                nc.gpsimd.tensor_add(
                    out=ot[:M, CSPLIT:W],
                    in0=mt[:M, CSPLIT:W],
                    in1=sw[:M, W + CSPLIT : 2 * W],
                )
                nc.sync.dma_start(out=y_d[b, r0 : r0 + M, :], in_=ot[:M])

            # Two-stage software pipeline: FRONT(si) = input prefetch,
            # matmuls, PSUM extraction, |t-c| args; BACK(si) = wide
            # sigmoid, combine, output DMA, emitted one iteration later.
            prefetch(0)
            back = [None]

            for si, (b, r0, M) in enumerate(strips):
                k = M + 2
                m0, m1 = bands[M]
                xt = xts.pop(si)
                prefetch(si + 1)

                ps = ppool.tile([STRIDE, W], f32, tag="ps")
                m0s = m0[:k, :M]
                m1s = m1[:k, :M]

                # Pre-touch: a 1x1 matmul absorbs the PSUM-release wait
                # (Matmult carries at most ONE sync wait; without this,
                # wait-merging couples strip t to strip t-1's consumers
                # and serializes PE behind them).
                nc.tensor.matmul(
                    ps[:1, 0:1], b5[:1, :1], b5[:1, :1], start=True, stop=True
                )

                # around = sum of 8 neighbors, accumulated in PSUM.
                for nb in range(NBANKS):
                    c0 = nb * 512
                    c1 = c0 + 512
                    nc.tensor.matmul(
                        ps[:M, c0:c1], m0s, xt[:k, c0:c1], start=True, stop=False
                    )
                    if nb == 0:
                        nc.tensor.matmul(
                            ps[:M, 1:512], m1s, xt[:k, 0:511],
                            start=False, stop=False,
                        )
                        nc.tensor.matmul(
                            ps[:M, 0:1], m1s, xt[:k, W - 1 : W],
                            start=False, stop=False,
                        )
                    else:
                        nc.tensor.matmul(
                            ps[:M, c0:c1], m1s, xt[:k, c0 - 1 : c1 - 1],
                            start=False, stop=False,
                        )
                    if nb == NBANKS - 1:
                        nc.tensor.matmul(
                            ps[:M, c0 : W - 1], m1s, xt[:k, c0 + 1 : W],
                            start=False, stop=False,
                        )
                        nc.tensor.matmul(
                            ps[:M, W - 1 : W], m1s, xt[:k, 0:1],
                            start=False, stop=True,
                        )
                    else:
                        nc.tensor.matmul(
                            ps[:M, c0:c1], m1s, xt[:k, c0 + 1 : c1 + 1],
                            start=False, stop=True,
                        )

                # zw = [t-2 | t-3] fp16.  Half 0 straight off PSUM on the
                # scalar engine (doubles as the PSUM->SBUF conversion),
                # half 1 derived on DVE at 4x rate.
                zw = zpool.tile([STRIDE, 2 * W], f16, tag="zw")
                nc.scalar.activation(zw[:M, 0:W], ps[:M], Cpy, bias=-2.0)
                nc.vector.tensor_scalar_sub(
                    out=zw[:M, W : 2 * W], in0=zw[:M, 0:W], scalar1=1.0
                )

                # zq = |zw| = max(zw, -zw): 4x negate + 2x max (the fused
                # scalar_tensor_tensor form only runs at 1x, so two ops
                # are faster)
                nzw = npool.tile([STRIDE, 2 * W], f16, tag="nzw")
                nc.vector.tensor_scalar_mul(out=nzw[:M], in0=zw[:M], scalar1=-1.0)
                zq = qpool.tile([STRIDE, 2 * W], f16, tag="zq")
                nc.vector.tensor_tensor(
                    out=zq[:M], in0=zw[:M], in1=nzw[:M], op=Alu.max
                )

                back.append((b, r0, M, xt, zq))
                prev = back.pop(0)
                if prev is not None:
                    emit_back(*prev)

            last = back.pop(0)
            if last is not None:
                emit_back(*last)

    nc.compile()
    _cached[key] = nc
    return nc


def run(x, trace=False):
    """Run the SPMD kernel on 8 cores. Returns (out_fp32, BassKernelResults)."""
    from concourse.bass_utils import run_bass_kernel_spmd

    nc = _build()
    x = np.asarray(x, dtype=np.float32)
    assert x.shape == (B, H, W), x.shape
    x16 = x.astype(np.float16)  # host-side cast: halves input HBM traffic
    in_maps = [{"x": x16[B_PER * c : B_PER * (c + 1)]} for c in range(N_CORES)]
    res = run_bass_kernel_spmd(nc, in_maps, core_ids=list(range(N_CORES)), trace=trace)
    out = np.concatenate(
        [res.results[c]["y"].astype(np.float32) for c in range(N_CORES)], axis=0
    )
    return out, res


def kernel(x):
    out, _ = run(x, trace=False)
    return out


# revision 12
# speedup vs baseline: 2.0560x; 1.0236x over previous
"""Continuous Game-of-Life Trainium2 kernel.

Reference computation (per batch image, cyclic 3x3 stencil):
    around = 8-neighbor sum of x (torus wrap)
    survive = sigmoid(10(around-1.5)) * sigmoid(10(3.5-around))
    birth   = sigmoid(10(around-2.5)) * sigmoid(10(3.5-around))
    out     = x*survive + (1-x)*birth

Algebra: with BETA=10 the transitions are >= 1.0 apart, so
    out ~= x*(s15-s25) + (s25-s35),  s_c = sigmoid(10*around-10c)
Both sigmoid differences are copies of the same unit bump
    g(u) = sigmoid(10u+5) - sigmoid(10u-5) = sigmoid(5-10|u|) + O(6.7e-3):
        s15-s25 = g(around-2),   s25-s35 = g(around-3)
so ONE double-width sigmoid pass over [|t-2| , |t-3|] replaces three
full-width sigmoid passes (ScalarE was the bottleneck: 3 passes
~204us/core -> extraction pass + double-width pass ~190us... split so
the extraction doubles as the PSUM->SBUF fp16 conversion).  Measured
rel err of the whole fp16 pipeline: ~2.4e-3 (harness gate 2e-2).

Memory: x is cast to fp16 on the HOST, so the device reads 16MB/core
instead of 32MB (the stencil and final multiply use fp16 x anyway).
In+out HBM traffic: 32MB/core.  Input stays on gpsimd SWDGE (its
descriptors fan out over all 16 DMA engines; a HWDGE DRAM->SBUF dma is
pinned to one engine at 22.5 GB/s) but no longer casts.

Engine mapping per 126-row strip of a 2048x2048 image:
  - DMA in (gpsimd SWDGE, fp16): 128 rows, prefetched one strip ahead.
  - TensorE: 8-neighbor sum via banded matmuls in PSUM (3 passes over W:
    no-center band on center columns + full band on column-shifted
    views; W-wrap via two 1-col edge matmuls).
  - ScalarE: zw0 = Copy(ps - 2) -> fp16 SBUF (= t-2; the PSUM read).
  - DVE: zw1 = zw0 - 1 (4x tensor_scalar)              (= t-3)
  - DVE: nzw = -zw (4x tensor_scalar, double-width)
  - DVE: zq = max(zw, nzw) (2x tensor_tensor, double-width) (= |t-c|)
  - ScalarE: ONE wide sigmoid sw = sigmoid(5 - 10*zq) -> [d1 | d2].
  - DVE: m = x*d1 (2x); out = m + d2 split DVE / GpSimd by columns.
  - DMA out (sync HWDGE): fp16, host upcasts to fp32.
The scalar engine's two ops per strip are software-pipelined (the wide
sigmoid of strip t-1 is emitted after the extraction of strip t) so
ScalarE never stalls on the intervening DVE ops of the same strip.

Sharding: data-parallel over batch: 16 images -> 8 cores x 2 images.
Torus wrap is per-image so there is no cross-core halo.
"""

import numpy as np

B, H, W = 16, 2048, 2048
N_CORES = 8
B_PER = B // N_CORES  # 2 images per core
STRIDE = 126  # output rows per strip (128 input rows incl. 1-row halos)
N_STRIPS = (H + STRIDE - 1) // STRIDE  # 17
NBANKS = W // 512  # PSUM banks per strip

_cached = {}


def _band_matrices(m, dtype=np.float16):
    """[m+2, m] stationary operands for the vertical taps.

    Tile layout: partitions 0..m-1 hold image rows r0..r0+m-1 (the cells),
    partition m holds the bottom halo row r0+m, partition m+1 holds the top
    halo row r0-1.  For output row p the vertical neighbors are partitions
    p-1 (or m+1 when p==0) and p+1.

    m0[k, p] = 1 for the two vertical neighbors (no center),
    m1[k, p] = 1 for the full 3-tap (used on the column-shifted views).
    """
    m0 = np.zeros((m + 2, m), dtype)
    m1 = np.zeros((m + 2, m), dtype)
    for p in range(m):
        up = m + 1 if p == 0 else p - 1
        m0[up, p] = 1.0
        m0[p + 1, p] = 1.0
        m1[up, p] = 1.0
        m1[p, p] = 1.0
        m1[p + 1, p] = 1.0
    return m0, m1


def _build():
    key = (B_PER, H, W, STRIDE)
    if key in _cached:
        return _cached[key]

    import concourse.mybir as mybir
    from concourse.bacc import Bacc
    from concourse.tile import TileContext

    KROWS = STRIDE + 2
    f32 = mybir.dt.float32
    f16 = mybir.dt.float16
    Sig = mybir.ActivationFunctionType.Sigmoid
    Cpy = mybir.ActivationFunctionType.Copy
    Alu = mybir.AluOpType

    nc = Bacc(trn_type="TRN2")
    x_d = nc.dram_tensor("x", [B_PER, H, W], f16, kind="ExternalInput")
    y_d = nc.dram_tensor("y", [B_PER, H, W], f16, kind="ExternalOutput")

    consts = {}
    for m in sorted({STRIDE, H - STRIDE * (N_STRIPS - 1)}):
        m0_np, m1_np = _band_matrices(m)
        consts[m] = (
            nc.inline_tensor(m0_np, f"m0_const_{m}"),
            nc.inline_tensor(m1_np, f"m1_const_{m}"),
        )

    strips = []
    for b in range(B_PER):
        for t in range(N_STRIPS):
            r0 = t * STRIDE
            strips.append((b, r0, min(STRIDE, H - r0)))

    with TileContext(nc) as tc:
        with (
            tc.tile_pool(name="wpool", bufs=1) as wpool,
            tc.tile_pool(name="xpool", bufs=6) as xpool,
            tc.tile_pool(name="zpool", bufs=3) as zpool,
            tc.tile_pool(name="npool", bufs=3) as npool,
            tc.tile_pool(name="qpool", bufs=3) as qpool,
            tc.tile_pool(name="spool", bufs=3) as spool,
            tc.tile_pool(name="mpool", bufs=3) as mpool,
            tc.tile_pool(name="opool", bufs=4) as opool,
            tc.tile_pool(name="ppool", bufs=2, space="PSUM") as ppool,
        ):
            bands = {}
            for m, (m0_d, m1_d) in consts.items():
                m0 = wpool.tile([m + 2, m], f16, name=f"m0_{m}")
                m1 = wpool.tile([m + 2, m], f16, name=f"m1_{m}")
                nc.sync.dma_start(out=m0[:], in_=m0_d[:])
                nc.sync.dma_start(out=m1[:], in_=m1_d[:])
                bands[m] = (m0, m1)

            # sigmoid bias must be an AP, not an immediate
            b5 = wpool.tile([128, 1], f32)
            nc.vector.memset(b5[:], 5.0)

            def issue_dma(b, r0, M, xt):
                # cells + bottom halo + top halo, fp16.  gpsimd SWDGE: its
                # descriptors fan out over all 16 DMA engines (a HWDGE
                # DRAM->SBUF dma is pinned to ONE engine, 22.5 GB/s).
                if r0 + M < H:
                    nc.gpsimd.dma_start(
                        out=xt[0 : M + 1, :], in_=x_d[b, r0 : r0 + M + 1, :]
                    )
                else:
                    nc.gpsimd.dma_start(out=xt[0:M, :], in_=x_d[b, r0:H, :])
                    nc.gpsimd.dma_start(out=xt[M : M + 1, :], in_=x_d[b, 0:1, :])
                rtop = (r0 - 1) % H
                nc.gpsimd.dma_start(
                    out=xt[M + 1 : M + 2, :], in_=x_d[b, rtop : rtop + 1, :]
                )

            xts = {}

            def prefetch(si):
                if si < len(strips):
                    pb, pr0, pM = strips[si]
                    t = xpool.tile([KROWS, W], f16, tag="xt", name=f"xt_{si}")
                    issue_dma(pb, pr0, pM, t)
                    xts[si] = t

            CSPLIT = 640  # add-columns done by DVE; the rest go to gpsimd

            def emit_back(b, r0, M, xt, zq):
                # ONE wide sigmoid: [d1 | d2] = sigmoid(5 - 10*|t-c|)
                sw = spool.tile([STRIDE, 2 * W], f16, tag="sw", name="sw")
                nc.scalar.activation(sw[:M], zq[:M], Sig, bias=b5[:M], scale=-10.0)
                # out = x*d1 + d2  (mul on DVE; add split DVE/gpsimd)
                mt = mpool.tile([STRIDE, W], f16, tag="m", name="mt")
                ot = opool.tile([STRIDE, W], f16, tag="o", name="ot")
                nc.vector.tensor_mul(out=mt[:M], in0=xt[:M, :], in1=sw[:M, 0:W])
                nc.vector.tensor_add(
                    out=ot[:M, 0:CSPLIT],
                    in0=mt[:M, 0:CSPLIT],
                    in1=sw[:M, W : W + CSPLIT],
                )
                nc.gpsimd.tensor_add(
                    out=ot[:M, CSPLIT:W],
                    in0=mt[:M, CSPLIT:W],
                    in1=sw[:M, W + CSPLIT : 2 * W],
                )
                nc.sync.dma_start(out=y_d[b, r0 : r0 + M, :], in_=ot[:M])

            # Two-stage software pipeline: FRONT(si) = input prefetch,
            # matmuls, PSUM extraction, |t-c| args; BACK(si) = wide
            # sigmoid, combine, output DMA, emitted one iteration later.
            prefetch(0)
            back = [None]

            for si, (b, r0, M) in enumerate(strips):
                k = M + 2
                m0, m1 = bands[M]
                xt = xts.pop(si)
                prefetch(si + 1)

                ps = ppool.tile([STRIDE, W], f32, tag="ps")
                m0s = m0[:k, :M]
                m1s = m1[:k, :M]

                # Pre-touch: a 1x1 matmul absorbs the PSUM-release wait
                # (Matmult carries at most ONE sync wait; without this,
                # wait-merging couples strip t to strip t-1's consumers
                # and serializes PE behind them).
                nc.tensor.matmul(
                    ps[:1, 0:1], b5[:1, :1], b5[:1, :1], start=True, stop=True
                )

                # around = sum of 8 neighbors, accumulated in PSUM.
                for nb in range(NBANKS):
                    c0 = nb * 512
                    c1 = c0 + 512
                    nc.tensor.matmul(
                        ps[:M, c0:c1], m0s, xt[:k, c0:c1], start=True, stop=False
                    )
                    if nb == 0:
                        nc.tensor.matmul(
                            ps[:M, 1:512], m1s, xt[:k, 0:511],
                            start=False, stop=False,
                        )
                        nc.tensor.matmul(
                            ps[:M, 0:1], m1s, xt[:k, W - 1 : W],
                            start=False, stop=False,
                        )
                    else:
                        nc.tensor.matmul(
                            ps[:M, c0:c1], m1s, xt[:k, c0 - 1 : c1 - 1],
                            start=False, stop=False,
                        )
                    if nb == NBANKS - 1:
                        nc.tensor.matmul(
                            ps[:M, c0 : W - 1], m1s, xt[:k, c0 + 1 : W],
                            start=False, stop=False,
                        )
                        nc.tensor.matmul(
                            ps[:M, W - 1 : W], m1s, xt[:k, 0:1],
                            start=False, stop=True,
                        )
                    else:
                        nc.tensor.matmul(
                            ps[:M, c0:c1], m1s, xt[:k, c0 + 1 : c1 + 1],
                            start=False, stop=True,
                        )

                # zw = [t-2 | t-3] fp16.  Half 0 straight off PSUM on the
                # scalar engine (doubles as the PSUM->SBUF conversion),
                # half 1 derived on DVE at 4x rate.
                zw = zpool.tile([STRIDE, 2 * W], f16, tag="zw")
                nc.scalar.activation(zw[:M, 0:W], ps[:M], Cpy, bias=-2.0)
                nc.vector.tensor_scalar_sub(
                    out=zw[:M, W : 2 * W], in0=zw[:M, 0:W], scalar1=1.0
                )

                # zq = |zw| = max(zw, -zw): 4x negate + 2x max (the fused
                # scalar_tensor_tensor form only runs at 1x, so two ops
                # are faster)
                nzw = npool.tile([STRIDE, 2 * W], f16, tag="nzw")
                nc.vector.tensor_scalar_mul(out=nzw[:M], in0=zw[:M], scalar1=-1.0)
                zq = qpool.tile([STRIDE, 2 * W], f16, tag="zq")
                nc.vector.tensor_tensor(
                    out=zq[:M], in0=zw[:M], in1=nzw[:M], op=Alu.max
                )

                back.append((b, r0, M, xt, zq))
                prev = back.pop(0)
                if prev is not None:
                    emit_back(*prev)

            last = back.pop(0)
            if last is not None:
                emit_back(*last)

    nc.compile()
    _cached[key] = nc
    return nc


def run(x, trace=False):
    """Run the SPMD kernel on 8 cores. Returns (out_fp32, BassKernelResults)."""
    from concourse.bass_utils import run_bass_kernel_spmd

    nc = _build()
    x = np.asarray(x, dtype=np.float32)
    assert x.shape == (B, H, W), x.shape
    x16 = x.astype(np.float16)  # host-side cast: halves input HBM traffic
    in_maps = [{"x": x16[B_PER * c : B_PER * (c + 1)]} for c in range(N_CORES)]
    res = run_bass_kernel_spmd(nc, in_maps, core_ids=list(range(N_CORES)), trace=trace)
    out = np.concatenate(
        [res.results[c]["y"].astype(np.float32) for c in range(N_CORES)], axis=0
    )
    return out, res


def kernel(x):
    out, _ = run(x, trace=False)
    return out
